# revision 1
# baseline (speedup 1.0000x reference)
"""Causal self-attention (RoPE) Trainium2 kernel, 8-way tensor-parallel.

Sharding (Megatron-style, zero-cost input distribution since every core
receives the full inputs): core c owns global heads {2c, 2c+1}.

Per core:
  1. qkv^T = W_slice^T @ x^T   (fp16 matmuls, fp32 psum), single pass over x^T;
     per token-chunk epilogue: bias add, RoPE on q/k (pair-swap DMA + 3 DVE
     ops), v PE-transposed to [token, d] layout
  2. Per (head, batch): causal flash-style attention
       scores psum [q,k] -> +mask -> exp (ACT, fused row-sum accum) ->
       normalize by 1/l (DVE) -> PE-transpose P-hat -> PV matmuls -> y^T [d,q]
  3. AllGather of y^T slices across the 8 cores, one per (head, batch),
     so projection work can start while later attention blocks still run
  4. Output projection vs the core's 256-column slice of W_proj in two
     accumulation halves (even heads -> fp32 partial in SBUF as soon as the
     even AGs land, odd heads + partial + bias -> fp32 out [tokens, 256])

Host side shards weights, builds RoPE/mask tables, and concatenates the
8 column slices into the final [B, T, C] output.
"""

import functools
import numpy as np

import concourse.bass as bass
import concourse.mybir as mybir
import concourse.tile as tile
from concourse import bacc
from concourse.bass_utils import run_bass_kernel_spmd
from concourse.masks import make_identity
from concourse.tile import add_dep_helper

F32 = mybir.dt.float32
F16 = mybir.dt.float16

N_CORES = 8
C = 2048           # model dim
H = 16             # total heads
HD = 128           # head dim
HL = 2             # heads per core
OC = C // N_CORES  # output cols per core (256)
SCALE = 1.0 / float(np.sqrt(HD))
MASK_VAL = -900.0  # additive pre-scale mask; exp arg ~ -80 -> underflows to 0


def build(B=2, T=2048, collective=True, n_cores=N_CORES):
    """Build the SPMD Bass program (identical on every core)."""
    BT = B * T
    NSTR = 3 * HL                  # qkv strips of 128 cols
    NCT = C // 128                 # contraction tiles
    NTCH = BT // 512               # token chunks for qkv
    NQC = T // 512                 # q chunks per (b, h)
    NTT = T // 128                 # token tiles per batch

    nc = bacc.Bacc(None, target_bir_lowering=False)
    xT = nc.dram_tensor("xT", [C, BT], F16, kind="ExternalInput")
    wqkv = nc.dram_tensor("wqkv", [C, NSTR * 128], F16, kind="ExternalInput")
    bqkv = nc.dram_tensor("bqkv", [NSTR * 128, 1], F32, kind="ExternalInput")
    ctil = nc.dram_tensor("ctil", [128, T], F16, kind="ExternalInput")
    stil = nc.dram_tensor("stil", [128, T], F16, kind="ExternalInput")
    wp = nc.dram_tensor("wp", [C, OC], F16, kind="ExternalInput")
    bpb = nc.dram_tensor("bpb", [128, OC], F32, kind="ExternalInput")
    cmask = nc.dram_tensor("cmask", [128, 128], F32, kind="ExternalInput")
    out = nc.dram_tensor("out", [BT, OC], F32, kind="ExternalOutput")

    with tile.TileContext(nc) as tc:
        with (
            tc.tile_pool(name="big", bufs=1) as big,
            tc.tile_pool(name="dram", bufs=1, space="DRAM") as dram,
        ):
            # ---- persistent SBUF tensors ----
            qr = big.tile([128, HL * BT], F16, tag="qr")
            kr = big.tile([128, HL * BT], F16, tag="kr")
            v_sb = big.tile([128, HL * BT], F16, tag="v_sb")
            ct_sb = big.tile([128, T], F16, tag="ct")
            st_sb = big.tile([128, T], F16, tag="st")
            ident = big.tile([128, 128], F16, tag="ident")
            cm_sb = big.tile([128, 128], F32, tag="cm")
            bq_sb = big.tile([128, NSTR], F32, tag="bq")
            bp_sb = big.tile([128, OC], F32, tag="bp")

            # DRAM bounce buffers: one AllGather per (local head j, batch b)
            # every (head, batch) gather is split into two column halves so each
            # projection half starts while the second half is still on the wire
            split_ag = T >= 1024
            nh = 2 if split_ag else 1
            hw_ = T // nh
            agin = {}
            agout = {}
            for j in range(HL):
                for b in range(B):
                    for h in range(nh):
                        agin[(j, b, h)] = dram.tile([128, hw_], F16,
                                                    name=f"agin{j}_{b}_{h}")
                        agout[(j, b, h)] = dram.tile([n_cores * 128, hw_], F16,
                                                     name=f"agout{j}_{b}_{h}")

            # ================= Phase A: QKV + RoPE + v-transpose =================
            with (
                tc.tile_pool(name="wq", bufs=1) as wq_pool,
                tc.tile_pool(name="xt", bufs=2) as xt_pool,
                tc.tile_pool(name="rope", bufs=2) as rope_pool,
                tc.tile_pool(name="stage", bufs=6) as stage_pool,
                tc.tile_pool(name="qkv_ps", bufs=3, space="PSUM") as qkv_ps,
                tc.tile_pool(name="vt_ps", bufs=4, space="PSUM") as vt_ps,
            ):
                # interleave first-chunk xT loads with the weight loads so the
                # first matmul group isn't queued behind 3 MB of weights
                w_sb = []
                xts_first = []
                for ctn in range(NCT):
                    xt_t = xt_pool.tile([128, 512], F16, tag=f"xt{ctn}",
                                        name=f"xt{ctn}")
                    nc.sync.dma_start(xt_t[:], xT[ctn * 128:(ctn + 1) * 128, 0:512])
                    xts_first.append(xt_t)
                    wt = wq_pool.tile([128, NSTR * 128], F16, tag=f"w{ctn}",
                                      name=f"w{ctn}")
                    nc.sync.dma_start(wt[:], wqkv[ctn * 128:(ctn + 1) * 128, :])
                    w_sb.append(wt)
                for s in range(NSTR):
                    nc.sync.dma_start(bq_sb[:, s:s + 1], bqkv[s * 128:(s + 1) * 128, :])
                make_identity(nc, ident[:])

                for tch in range(NTCH):
                    if tch == min(1, NTCH - 1):
                        # constants land after the first xT burst is in flight
                        nc.sync.dma_start(ct_sb[:], ctil[:, :])
                        nc.sync.dma_start(st_sb[:], stil[:, :])
                        nc.sync.dma_start(cm_sb[:], cmask[:, :])
                        nc.sync.dma_start(bp_sb[:], bpb[:, :])
                    tw = (tch * 512) % T        # token offset within batch
                    tok = slice(tw, tw + 512)
                    if tch == 0:
                        xts = xts_first
                    else:
                        xts = []
                        for ctn in range(NCT):
                            xt_t = xt_pool.tile([128, 512], F16, tag=f"xt{ctn}",
                                                name=f"xt{ctn}")
                            nc.sync.dma_start(
                                xt_t[:],
                                xT[ctn * 128:(ctn + 1) * 128,
                                   tch * 512:(tch + 1) * 512])
                            xts.append(xt_t)
                    stg = {}
                    for s in range(NSTR):
                        ps = qkv_ps.tile([128, 512], F32, name="qkvps")
                        for ctn in range(NCT):
                            nc.tensor.matmul(
                                ps[:], w_sb[ctn][:, s * 128:(s + 1) * 128], xts[ctn][:],
                                start=(ctn == 0), stop=(ctn == NCT - 1))
                        kindtag = ("qs0", "qs1", "ks0", "ks1", "vs0", "vs1")[s]
                        st_t = stage_pool.tile([128, 512], F16, tag=kindtag,
                                               name=kindtag)
                        nc.scalar.activation(
                            st_t[:], ps[:], mybir.ActivationFunctionType.Identity,
                            bias=bq_sb[:, s:s + 1], scale=1.0)
                        stg[s] = st_t
                    for j in range(HL):
                        # RoPE on q and k for this (j, tch)
                        for st_t, dst, swtag in ((stg[j], qr, f"swq{j}"),
                                                 (stg[2 + j], kr, f"swk{j}")):
                            dstsl = dst[:, j * BT + tch * 512: j * BT + (tch + 1) * 512]
                            sw = rope_pool.tile([128, 512], F16, tag=swtag, name=swtag)
                            nc.sync.dma_start(sw[0:127:2, :], st_t[1:128:2, :])
                            nc.sync.dma_start(sw[1:128:2, :], st_t[0:127:2, :])
                            tmp = rope_pool.tile([128, 512], F16, tag=swtag + "t",
                                                 name=swtag + "t")
                            nc.vector.tensor_mul(dstsl, st_t[:], ct_sb[:, tok])
                            nc.vector.tensor_mul(tmp[:], sw[:], st_sb[:, tok])
                            nc.vector.tensor_add(dstsl, dstsl, tmp[:])
                        # v transpose for this (j, tch)
                        for blk in range(4):
                            tt = tch * 4 + blk
                            ps = vt_ps.tile([128, 128], F16, tag="vtp", name="vtp")
                            nc.tensor.transpose(
                                ps[:], stg[4 + j][:, blk * 128:(blk + 1) * 128],
                                ident[:])
                            nc.scalar.copy(
                                v_sb[:, j * BT + tt * 128: j * BT + (tt + 1) * 128],
                                ps[:])

            # ============ Phase B: attention + AG + interleaved projection ============
            with (
                tc.tile_pool(name="p", bufs=4) as p_pool,
                tc.tile_pool(name="pt", bufs=3) as pt_pool,
                tc.tile_pool(name="stat", bufs=16) as stat_pool,
                tc.tile_pool(name="yts", bufs=4) as yts_pool,
                tc.tile_pool(name="wpp", bufs=1) as wp_pool,
                tc.tile_pool(name="ygs", bufs=2) as ygs_pool,
                tc.tile_pool(name="part", bufs=1) as part_pool,
                tc.tile_pool(name="ot", bufs=4) as ot_pool,
                tc.tile_pool(name="sc_ps", bufs=2, space="PSUM") as sc_ps,
                tc.tile_pool(name="tp_ps", bufs=2, space="PSUM") as tp_ps,
                tc.tile_pool(name="y_ps", bufs=2, space="PSUM") as y_ps,
                tc.tile_pool(name="o_ps", bufs=1, space="PSUM") as o_ps,
            ):
                wp_sb = []
                for g in range(H):
                    wt = wp_pool.tile([128, OC], F16, tag=f"wp{g}", name=f"wp{g}")
                    nc.sync.dma_start(wt[:], wp[g * 128:(g + 1) * 128, :])
                    wp_sb.append(wt)
                partial = {}
                cc_insts = {}
                agin_dmas = {}

                def attention_block(j, b):
                    base = j * BT + b * T
                    for qc in range(NQC):
                        nkt = 4 * qc + 4
                        ptall = pt_pool.tile([128, 8192], F16, tag="ptall",
                                             name="ptall")
                        for qtw in range(4):
                            qt = qc * 4 + qtw
                            kext = (qt + 1) * 128
                            qtile = qr[:, base + qt * 128: base + (qt + 1) * 128]
                            ptile = p_pool.tile([128, T], F16, tag="P", name="P")
                            lparts = []
                            off = 0
                            while off < kext:
                                n = min(512, kext - off)
                                ps = sc_ps.tile([128, 512], F32, name="scps")
                                nc.tensor.matmul(
                                    ps[:, :n], qtile, kr[:, base + off: base + off + n],
                                    start=True, stop=True)
                                if off + n == kext:
                                    nc.vector.tensor_add(
                                        ps[:, n - 128:n], ps[:, n - 128:n], cm_sb[:])
                                lp = stat_pool.tile([128, 1], F32, tag="lp", name="lp")
                                nc.scalar.activation(
                                    ptile[:, off:off + n], ps[:, :n],
                                    mybir.ActivationFunctionType.Exp,
                                    scale=SCALE, accum_out=lp[:])
                                lparts.append(lp)
                                off += n
                            lsum = stat_pool.tile([128, 1], F32, tag="ls", name="ls")
                            if len(lparts) == 1:
                                lsum = lparts[0]
                            else:
                                nc.vector.tensor_add(lsum[:], lparts[0][:], lparts[1][:])
                                for lp in lparts[2:]:
                                    nc.vector.tensor_add(lsum[:], lsum[:], lp[:])
                            rec = stat_pool.tile([128, 1], F32, tag="rec", name="rec")
                            nc.vector.reciprocal(rec[:], lsum[:])
                            nc.vector.tensor_scalar_mul(
                                ptile[:, :kext], ptile[:, :kext], rec[:])
                            kt = 0
                            while kt <= qt:
                                if kt + 1 <= qt:
                                    tp = tp_ps.tile([128, 256], F16, tag="tp", name="tp")
                                    nc.tensor.transpose(
                                        tp[:, 0:128],
                                        ptile[:, kt * 128:(kt + 1) * 128], ident[:])
                                    nc.tensor.transpose(
                                        tp[:, 128:256],
                                        ptile[:, (kt + 1) * 128:(kt + 2) * 128],
                                        ident[:])
                                    dst = ptall[:].rearrange(
                                        "p (a b) -> p a b", b=512)[
                                        :, kt:kt + 2, qtw * 128:(qtw + 1) * 128]
                                    nc.vector.tensor_copy(
                                        dst,
                                        tp[:].rearrange("p (a b) -> p a b", a=2))
                                    kt += 2
                                else:
                                    tp = tp_ps.tile([128, 256], F16, tag="tp", name="tp")
                                    nc.tensor.transpose(
                                        tp[:, 0:128],
                                        ptile[:, kt * 128:(kt + 1) * 128], ident[:])
                                    nc.vector.tensor_copy(
                                        ptall[:, kt * 512 + qtw * 128:
                                              kt * 512 + (qtw + 1) * 128],
                                        tp[:, 0:128])
                                    kt += 1
                        psy = y_ps.tile([128, 512], F32, name="psy")
                        for kt in range(nkt):
                            qstart = max(0, (kt - 4 * qc)) * 128
                            nc.tensor.matmul(
                                psy[:, qstart:512],
                                v_sb[:, base + kt * 128: base + (kt + 1) * 128],
                                ptall[:, kt * 512 + qstart: kt * 512 + 512],
                                start=(kt == 0), stop=(kt == nkt - 1))
                        yt = yts_pool.tile([128, 512], F16, tag="yt", name="yt")
                        nc.vector.tensor_copy(yt[:], psy[:])
                        hqc = max(1, NQC // nh)
                        dstb = agin[(j, b, qc // hqc)]
                        dst_ap = dstb[:, (qc % hqc) * 512:(qc % hqc + 1) * 512]
                        d = nc.sync.dma_start(dst_ap, yt[:])
                        agin_dmas.setdefault((j, b), []).append(d)
                    ccs = []
                    for h in range(nh):
                        if collective:
                            cc = nc.gpsimd.collective_compute(
                                "AllGather",
                                mybir.AluOpType.bypass,
                                replica_groups=[list(range(n_cores))],
                                ins=[agin[(j, b, h)].opt()],
                                outs=[agout[(j, b, h)].opt()],
                            )
                        else:
                            cc = nc.sync.dma_start(agout[(j, b, h)][0:128, :],
                                                   agin[(j, b, h)][:, :])
                        cci = cc.ins if hasattr(cc, "ins") else cc
                        ndep = len(agin_dmas[(j, b)]) // nh
                        for d in agin_dmas[(j, b)][h * ndep:(h + 1) * ndep]:
                            di = d.ins if hasattr(d, "ins") else d
                            add_dep_helper(cci, di,
                                           reason="collective reads agin after y")
                        ccs.append(cci)
                    cc_insts[(j, b)] = ccs

                def proj_half(b, par, first):
                    """Accumulate heads of parity `par` for batch b.

                    first=True: psum + bias -> fp32 partial tiles in SBUF.
                    first=False: psum + partial -> output DMA.
                    """
                    gs = list(range(par, H, 2))
                    for tg0 in range(0, NTT, 4):
                        ng = min(4, NTT - tg0)
                        ygq = {}
                        for g in gs:
                            row = (g // 2) * 128
                            yg = ygs_pool.tile([128, ng * 128], F16, tag=f"yg{g}",
                                               name=f"yg{g}")
                            h = (tg0 * 128) // hw_
                            src_ap = agout[(g % 2, b, h)][
                                row:row + 128,
                                tg0 * 128 - h * hw_:(tg0 + ng) * 128 - h * hw_]
                            ccdep = cc_insts[(g % 2, b)][h]
                            d = nc.sync.dma_start(yg[:], src_ap)
                            di = d.ins if hasattr(d, "ins") else d
                            add_dep_helper(di, ccdep,
                                           reason="proj reads agout after collective")
                            ygq[g] = yg
                        for p0 in range(0, ng, 2):
                            npair = min(2, ng - p0)
                            pss = [o_ps.tile([128, OC], F32, tag=f"op{i}",
                                             name=f"op{i}")
                                   for i in range(npair)]
                            for gi, g in enumerate(gs):
                                for i in range(npair):
                                    nc.tensor.matmul(
                                        pss[i][:],
                                        ygq[g][:, (p0 + i) * 128:(p0 + i + 1) * 128],
                                        wp_sb[g][:],
                                        start=(gi == 0), stop=(gi == len(gs) - 1))
                            for i in range(npair):
                                tt = tg0 + p0 + i
                                if first:
                                    pt_t = part_pool.tile([128, OC], F32,
                                                          tag=f"part{b}_{tt}",
                                                          name=f"part{b}_{tt}")
                                    nc.vector.tensor_add(pt_t[:], pss[i][:], bp_sb[:])
                                    partial[(b, tt)] = pt_t
                                else:
                                    ot = ot_pool.tile([128, OC], F32, tag="ot",
                                                      name="ot")
                                    nc.vector.tensor_add(
                                        ot[:], pss[i][:], partial[(b, tt)][:])
                                    r0 = b * T + tt * 128
                                    nc.sync.dma_start(out[r0:r0 + 128, :], ot[:])

                for b in range(B):
                    attention_block(0, b)
                for b in range(B):
                    proj_half(b, 0, first=True)
                for b in range(B):
                    attention_block(1, b)
                    proj_half(b, 1, first=False)
    nc.compile()
    return nc


def _prep_inputs(x, W_attn, b_attn, W_proj, b_proj, cos, sin, core, B, T):
    """Host-side shard prep for one core."""
    BT = B * T
    xT = np.ascontiguousarray(x.reshape(BT, C).T).astype(np.float16)

    cols = []
    bvals = []
    for part in range(3):  # q, k, v
        for j in range(HL):
            h = 2 * core + j
            sl = slice(part * C + h * HD, part * C + (h + 1) * HD)
            cols.append(W_attn[:, sl])
            bvals.append(b_attn[sl])
    wqkv = np.concatenate(cols, axis=1).astype(np.float16)
    bqkv = np.concatenate(bvals).astype(np.float32).reshape(-1, 1)

    # RoPE tables: ctil[p, t] = cos[t, p//2]; stil[2i] = -sin, stil[2i+1] = +sin
    cosr = np.repeat(cos.T, 2, axis=0)  # [128, T]
    sinr = np.repeat(sin.T, 2, axis=0)
    sgn = np.where((np.arange(128) % 2) == 0, -1.0, 1.0)[:, None]
    ctil = cosr.astype(np.float16)
    stil = (sinr * sgn).astype(np.float16)

    wp_c = W_proj[:, core * OC:(core + 1) * OC].astype(np.float16)
    bpb = np.tile(b_proj[core * OC:(core + 1) * OC].astype(np.float32), (128, 1))
    ii, jj = np.mgrid[0:128, 0:128]
    cmask = np.where(jj <= ii, 0.0, MASK_VAL).astype(np.float32)
    return {
        "xT": xT, "wqkv": wqkv, "bqkv": bqkv, "ctil": ctil, "stil": stil,
        "wp": wp_c, "bpb": bpb, "cmask": cmask,
    }


@functools.lru_cache(maxsize=2)
def _built(B, T):
    return build(B=B, T=T)


_warmed = set()


def kernel(x, W_attn, b_attn, W_proj, b_proj, cos, sin):
    x = np.asarray(x, dtype=np.float32)
    W_attn = np.asarray(W_attn, dtype=np.float32)
    b_attn = np.asarray(b_attn, dtype=np.float32)
    W_proj = np.asarray(W_proj, dtype=np.float32)
    b_proj = np.asarray(b_proj, dtype=np.float32)
    cos = np.asarray(cos, dtype=np.float32)
    sin = np.asarray(sin, dtype=np.float32)

    B, T, Cv = x.shape
    assert Cv == C
    nc = _built(B, T)
    in_maps = [_prep_inputs(x, W_attn, b_attn, W_proj, b_proj, cos, sin, c, B, T)
               for c in range(N_CORES)]
    if (B, T) not in _warmed:
        # The very first execution of a freshly loaded NEFF has been observed
        # to deliver stale/uninitialized collective buffers; run once and
        # discard, then run for real.
        run_bass_kernel_spmd(nc, in_maps, core_ids=list(range(N_CORES)))
        _warmed.add((B, T))
    res = run_bass_kernel_spmd(nc, in_maps, core_ids=list(range(N_CORES)))
    outs = [res.results[c]["out"] for c in range(N_CORES)]
    full = np.concatenate(outs, axis=1)  # [BT, C]
    return full.reshape(B, T, C).astype(np.float32)



# revision 4
# speedup vs baseline: 1.0176x; 1.0176x over previous
"""Causal self-attention (RoPE) Trainium2 kernel, 8-way tensor-parallel.

Sharding (Megatron-style, zero-cost input distribution since every core
receives the full inputs): core c owns global heads {2c, 2c+1}.

Per core:
  1. qkv^T = W_slice^T @ x^T   (fp16 matmuls, fp32 psum), single pass over x^T;
     per token-chunk epilogue: bias add, RoPE on q/k (pair-swap DMA + 3 DVE
     ops), v PE-transposed to [token, d] layout
  2. Per (head, batch): causal flash-style attention
       scores psum [q,k] -> +mask -> exp (ACT, fused row-sum accum) ->
       normalize by 1/l (DVE) -> PE-transpose P-hat -> PV matmuls -> y^T [d,q]
  3. AllGather of y^T slices across the 8 cores, one per (head, batch),
     so projection work can start while later attention blocks still run
  4. Output projection vs the core's 256-column slice of W_proj in two
     accumulation halves (even heads -> fp32 partial in SBUF as soon as the
     even AGs land, odd heads + partial + bias -> fp32 out [tokens, 256])

Host side shards weights, builds RoPE/mask tables, and concatenates the
8 column slices into the final [B, T, C] output.
"""

import functools
import numpy as np

import concourse.bass as bass
import concourse.mybir as mybir
import concourse.tile as tile
from concourse import bacc
from concourse.bass_utils import run_bass_kernel_spmd
from concourse.masks import make_identity
from concourse.tile import add_dep_helper

F32 = mybir.dt.float32
F16 = mybir.dt.float16

N_CORES = 8
C = 2048           # model dim
H = 16             # total heads
HD = 128           # head dim
HL = 2             # heads per core
OC = C // N_CORES  # output cols per core (256)
SCALE = 1.0 / float(np.sqrt(HD))
MASK_VAL = -900.0  # additive pre-scale mask; exp arg ~ -80 -> underflows to 0


def build(B=2, T=2048, collective=True, n_cores=N_CORES):
    """Build the SPMD Bass program (identical on every core)."""
    BT = B * T
    NSTR = 3 * HL                  # qkv strips of 128 cols
    NCT = C // 128                 # contraction tiles
    NTCH = BT // 512               # token chunks for qkv
    NQC = T // 512                 # q chunks per (b, h)
    NTT = T // 128                 # token tiles per batch

    nc = bacc.Bacc(None, target_bir_lowering=False)
    xT = nc.dram_tensor("xT", [C, BT], F16, kind="ExternalInput")
    wqkv = nc.dram_tensor("wqkv", [C, NSTR * 128], F16, kind="ExternalInput")
    bqkv = nc.dram_tensor("bqkv", [NSTR * 128, 1], F32, kind="ExternalInput")
    ctil = nc.dram_tensor("ctil", [128, T], F16, kind="ExternalInput")
    stil = nc.dram_tensor("stil", [128, T], F16, kind="ExternalInput")
    wp = nc.dram_tensor("wp", [C, OC], F16, kind="ExternalInput")
    bpb = nc.dram_tensor("bpb", [128, OC], F32, kind="ExternalInput")
    cmask = nc.dram_tensor("cmask", [128, 128], F32, kind="ExternalInput")
    out = nc.dram_tensor("out", [BT, OC], F32, kind="ExternalOutput")

    with tile.TileContext(nc) as tc:
        with (
            tc.tile_pool(name="big", bufs=1) as big,
            tc.tile_pool(name="dram", bufs=1, space="DRAM") as dram,
        ):
            # ---- persistent SBUF tensors ----
            qr = big.tile([128, HL * BT], F16, tag="qr")
            kr = big.tile([128, HL * BT], F16, tag="kr")
            v_sb = big.tile([128, HL * BT], F16, tag="v_sb")
            ct_sb = big.tile([128, T], F16, tag="ct")
            st_sb = big.tile([128, T], F16, tag="st")
            ident = big.tile([128, 128], F16, tag="ident")
            cm_sb = big.tile([128, 128], F32, tag="cm")
            bq_sb = big.tile([128, NSTR], F32, tag="bq")
            bp_sb = big.tile([128, OC], F32, tag="bp")

            # DRAM bounce buffers: one AllGather per (local head j, batch b)
            # every (head, batch) gather is split into two column halves so each
            # projection half starts while the second half is still on the wire
            split_ag = T >= 1024
            nh = 2 if split_ag else 1
            hw_ = T // nh
            agin = {}
            agout = {}
            for j in range(HL):
                for b in range(B):
                    for h in range(nh):
                        agin[(j, b, h)] = dram.tile([128, hw_], F16,
                                                    name=f"agin{j}_{b}_{h}")
                        agout[(j, b, h)] = dram.tile([n_cores * 128, hw_], F16,
                                                     name=f"agout{j}_{b}_{h}")

            # ================= Phase A: QKV + RoPE + v-transpose =================
            with (
                tc.tile_pool(name="wq", bufs=1) as wq_pool,
                tc.tile_pool(name="xt", bufs=2) as xt_pool,
                tc.tile_pool(name="rope", bufs=2) as rope_pool,
                tc.tile_pool(name="stage", bufs=2) as stage_pool,
                tc.tile_pool(name="qkv_ps", bufs=3, space="PSUM") as qkv_ps,
                tc.tile_pool(name="vt_ps", bufs=4, space="PSUM") as vt_ps,
            ):
                # Merged multi-tile DMAs: one SBUF tile holds all NCT 128-row
                # blocks side by side; a single DMA with a 3-d access pattern
                # fills it. Quarters of the weight tensor interleave with the
                # first-chunk x halves so the first matmuls start early.
                w_all = wq_pool.tile([128, NCT * NSTR * 128], F16, tag="wall",
                                     name="wall")

                def w_slice(ctn, s):
                    off = ctn * NSTR * 128 + s * 128
                    return w_all[:, off:off + 128]

                def load_xt_chunk(tch, ctn0, ctn1, xt_t=None):
                    if xt_t is None:
                        xt_t = xt_pool.tile([128, NCT * 512], F16, tag="xtch",
                                            name="xtch")
                    src = xT[ctn0 * 128:ctn1 * 128,
                             tch * 512:(tch + 1) * 512]
                    nc.sync.dma_start(
                        xt_t[:, ctn0 * 512:ctn1 * 512].rearrange(
                            "p (c t) -> p c t", t=512),
                        src.rearrange("(c p) t -> p c t", p=128))
                    return xt_t

                def load_w(c0, c1):
                    nc.sync.dma_start(
                        w_all[:, c0 * NSTR * 128:c1 * NSTR * 128]
                        .rearrange("p (c t) -> p c t", t=NSTR * 128),
                        wqkv[c0 * 128:c1 * 128, :]
                        .rearrange("(c p) t -> p c t", p=128))

                # first chunk: x half, w quarters interleaved, x half, w half
                xt_first = load_xt_chunk(0, 0, NCT // 2)
                load_w(0, 4)
                load_w(4, 8)
                load_xt_chunk(0, NCT // 2, NCT, xt_t=xt_first)
                load_w(8, NCT)
                nc.sync.dma_start(
                    bq_sb[:, 0:NSTR].rearrange("p (c t) -> p c t", t=1),
                    bqkv[:, :].rearrange("(c p) t -> p c t", p=128))
                make_identity(nc, ident[:])

                for tch in range(NTCH):
                    if tch == min(1, NTCH - 1):
                        # constants land after the first xT burst is in flight
                        nc.sync.dma_start(ct_sb[:], ctil[:, :])
                        nc.sync.dma_start(st_sb[:], stil[:, :])
                        nc.sync.dma_start(cm_sb[:], cmask[:, :])
                        nc.sync.dma_start(bp_sb[:], bpb[:, :])
                    tw = (tch * 512) % T        # token offset within batch
                    tok = slice(tw, tw + 512)
                    if tch == 0:
                        xts = xt_first
                    else:
                        xts = load_xt_chunk(tch, 0, NCT)
                    # combined q/k staging tile so the RoPE pair-swap is two
                    # strided DMAs per chunk instead of two per strip
                    qkstage = stage_pool.tile([128, 4 * 512], F16, tag="qkst",
                                              name="qkst")
                    vstg = {}
                    for s in range(NSTR):
                        ps = qkv_ps.tile([128, 512], F32, name="qkvps")
                        for ctn in range(NCT):
                            nc.tensor.matmul(
                                ps[:], w_slice(ctn, s),
                                xts[:, ctn * 512:(ctn + 1) * 512],
                                start=(ctn == 0), stop=(ctn == NCT - 1))
                        if s < 4:
                            st_t = qkstage[:, s * 512:(s + 1) * 512]
                        else:
                            st_t = stage_pool.tile([128, 512], F16,
                                                   tag=f"vs{s - 4}",
                                                   name=f"vs{s - 4}")
                            vstg[s - 4] = st_t
                        nc.scalar.activation(
                            st_t[:], ps[:], mybir.ActivationFunctionType.Identity,
                            bias=bq_sb[:, s:s + 1], scale=1.0)
                    # pair-swap of the whole q/k staging block (2 strided DMAs)
                    sw = rope_pool.tile([128, 4 * 512], F16, tag="swqk",
                                        name="swqk")
                    nc.sync.dma_start(sw[0:127:2, :], qkstage[1:128:2, :])
                    nc.sync.dma_start(sw[1:128:2, :], qkstage[0:127:2, :])
                    for j in range(HL):
                        # RoPE on q and k for this (j, tch)
                        for part, dst in ((j, qr), (2 + j, kr)):
                            st_t = qkstage[:, part * 512:(part + 1) * 512]
                            sw_t = sw[:, part * 512:(part + 1) * 512]
                            dstsl = dst[:, j * BT + tch * 512: j * BT + (tch + 1) * 512]
                            tmp = rope_pool.tile([128, 512], F16, tag=f"rt{part}",
                                                 name=f"rt{part}")
                            nc.vector.tensor_mul(dstsl, st_t, ct_sb[:, tok])
                            nc.vector.tensor_mul(tmp[:], sw_t, st_sb[:, tok])
                            nc.vector.tensor_add(dstsl, dstsl, tmp[:])
                        # v transpose for this (j, tch)
                        for blk in range(4):
                            tt = tch * 4 + blk
                            ps = vt_ps.tile([128, 128], F16, tag="vtp", name="vtp")
                            nc.tensor.transpose(
                                ps[:], vstg[j][:, blk * 128:(blk + 1) * 128],
                                ident[:])
                            nc.scalar.copy(
                                v_sb[:, j * BT + tt * 128: j * BT + (tt + 1) * 128],
                                ps[:])

            # ============ Phase B: attention + AG + interleaved projection ============
            with (
                tc.tile_pool(name="p", bufs=4) as p_pool,
                tc.tile_pool(name="pt", bufs=3) as pt_pool,
                tc.tile_pool(name="stat", bufs=16) as stat_pool,
                tc.tile_pool(name="yts", bufs=4) as yts_pool,
                tc.tile_pool(name="wpp", bufs=1) as wp_pool,
                tc.tile_pool(name="ygs", bufs=2) as ygs_pool,
                tc.tile_pool(name="part", bufs=1) as part_pool,
                tc.tile_pool(name="ot", bufs=4) as ot_pool,
                tc.tile_pool(name="sc_ps", bufs=2, space="PSUM") as sc_ps,
                tc.tile_pool(name="tp_ps", bufs=2, space="PSUM") as tp_ps,
                tc.tile_pool(name="y_ps", bufs=2, space="PSUM") as y_ps,
                tc.tile_pool(name="o_ps", bufs=1, space="PSUM") as o_ps,
            ):
                wp_sb = []
                for g in range(H):
                    wt = wp_pool.tile([128, OC], F16, tag=f"wp{g}", name=f"wp{g}")
                    nc.sync.dma_start(wt[:], wp[g * 128:(g + 1) * 128, :])
                    wp_sb.append(wt)
                partial = {}
                cc_insts = {}
                agin_dmas = {}

                def attention_block(j, b):
                    base = j * BT + b * T
                    for qc in range(NQC):
                        nkt = 4 * qc + 4
                        ptall = pt_pool.tile([128, 8192], F16, tag="ptall",
                                             name="ptall")
                        for qtw in range(4):
                            qt = qc * 4 + qtw
                            kext = (qt + 1) * 128
                            qtile = qr[:, base + qt * 128: base + (qt + 1) * 128]
                            ptile = p_pool.tile([128, T], F16, tag="P", name="P")
                            lparts = []
                            off = 0
                            while off < kext:
                                n = min(512, kext - off)
                                ps = sc_ps.tile([128, 512], F32, name="scps")
                                nc.tensor.matmul(
                                    ps[:, :n], qtile, kr[:, base + off: base + off + n],
                                    start=True, stop=True)
                                if off + n == kext:
                                    nc.vector.tensor_add(
                                        ps[:, n - 128:n], ps[:, n - 128:n], cm_sb[:])
                                lp = stat_pool.tile([128, 1], F32, tag="lp", name="lp")
                                nc.scalar.activation(
                                    ptile[:, off:off + n], ps[:, :n],
                                    mybir.ActivationFunctionType.Exp,
                                    scale=SCALE, accum_out=lp[:])
                                lparts.append(lp)
                                off += n
                            lsum = stat_pool.tile([128, 1], F32, tag="ls", name="ls")
                            if len(lparts) == 1:
                                lsum = lparts[0]
                            else:
                                nc.vector.tensor_add(lsum[:], lparts[0][:], lparts[1][:])
                                for lp in lparts[2:]:
                                    nc.vector.tensor_add(lsum[:], lsum[:], lp[:])
                            rec = stat_pool.tile([128, 1], F32, tag="rec", name="rec")
                            nc.vector.reciprocal(rec[:], lsum[:])
                            nc.vector.tensor_scalar_mul(
                                ptile[:, :kext], ptile[:, :kext], rec[:])
                            kt = 0
                            while kt <= qt:
                                if kt + 1 <= qt:
                                    tp = tp_ps.tile([128, 256], F16, tag="tp", name="tp")
                                    nc.tensor.transpose(
                                        tp[:, 0:128],
                                        ptile[:, kt * 128:(kt + 1) * 128], ident[:])
                                    nc.tensor.transpose(
                                        tp[:, 128:256],
                                        ptile[:, (kt + 1) * 128:(kt + 2) * 128],
                                        ident[:])
                                    dst = ptall[:].rearrange(
                                        "p (a b) -> p a b", b=512)[
                                        :, kt:kt + 2, qtw * 128:(qtw + 1) * 128]
                                    nc.vector.tensor_copy(
                                        dst,
                                        tp[:].rearrange("p (a b) -> p a b", a=2))
                                    kt += 2
                                else:
                                    tp = tp_ps.tile([128, 256], F16, tag="tp", name="tp")
                                    nc.tensor.transpose(
                                        tp[:, 0:128],
                                        ptile[:, kt * 128:(kt + 1) * 128], ident[:])
                                    nc.vector.tensor_copy(
                                        ptall[:, kt * 512 + qtw * 128:
                                              kt * 512 + (qtw + 1) * 128],
                                        tp[:, 0:128])
                                    kt += 1
                        psy = y_ps.tile([128, 512], F32, name="psy")
                        for kt in range(nkt):
                            qstart = max(0, (kt - 4 * qc)) * 128
                            nc.tensor.matmul(
                                psy[:, qstart:512],
                                v_sb[:, base + kt * 128: base + (kt + 1) * 128],
                                ptall[:, kt * 512 + qstart: kt * 512 + 512],
                                start=(kt == 0), stop=(kt == nkt - 1))
                        yt = yts_pool.tile([128, 512], F16, tag="yt", name="yt")
                        nc.vector.tensor_copy(yt[:], psy[:])
                        hqc = max(1, NQC // nh)
                        dstb = agin[(j, b, qc // hqc)]
                        dst_ap = dstb[:, (qc % hqc) * 512:(qc % hqc + 1) * 512]
                        d = nc.sync.dma_start(dst_ap, yt[:])
                        agin_dmas.setdefault((j, b), []).append(d)
                    ccs = []
                    for h in range(nh):
                        if collective:
                            cc = nc.gpsimd.collective_compute(
                                "AllGather",
                                mybir.AluOpType.bypass,
                                replica_groups=[list(range(n_cores))],
                                ins=[agin[(j, b, h)].opt()],
                                outs=[agout[(j, b, h)].opt()],
                            )
                        else:
                            cc = nc.sync.dma_start(agout[(j, b, h)][0:128, :],
                                                   agin[(j, b, h)][:, :])
                        cci = cc.ins if hasattr(cc, "ins") else cc
                        ndep = len(agin_dmas[(j, b)]) // nh
                        for d in agin_dmas[(j, b)][h * ndep:(h + 1) * ndep]:
                            di = d.ins if hasattr(d, "ins") else d
                            add_dep_helper(cci, di,
                                           reason="collective reads agin after y")
                        ccs.append(cci)
                    cc_insts[(j, b)] = ccs

                def proj_half(b, par, first):
                    """Accumulate heads of parity `par` for batch b.

                    first=True: psum + bias -> fp32 partial tiles in SBUF.
                    first=False: psum + partial -> output DMA.

                    All 8 heads of one parity live in the same agout buffer
                    (rows g//2*128), so one merged DMA loads the whole
                    [1024 x ng*128] block per token group.
                    """
                    gs = list(range(par, H, 2))
                    for tg0 in range(0, NTT, 4):
                        ng = min(4, NTT - tg0)
                        h = (tg0 * 128) % T // hw_
                        ygall = ygs_pool.tile([128, 8 * ng * 128], F16,
                                              tag="ygall", name="ygall")
                        src_ap = agout[(par, b, h)][
                            :, tg0 * 128 - h * hw_:(tg0 + ng) * 128 - h * hw_]
                        ccdep = cc_insts[(par, b)][h]
                        d = nc.sync.dma_start(
                            ygall[:].rearrange("p (blk t) -> p blk t",
                                               t=ng * 128),
                            src_ap.rearrange("(blk p) t -> p blk t", p=128))
                        di = d.ins if hasattr(d, "ins") else d
                        add_dep_helper(di, ccdep,
                                       reason="proj reads agout after collective")

                        def ygq(g, i):
                            blk = g // 2
                            off = blk * ng * 128 + i * 128
                            return ygall[:, off:off + 128]

                        for p0 in range(0, ng, 2):
                            npair = min(2, ng - p0)
                            pss = [o_ps.tile([128, OC], F32, tag=f"op{i}",
                                             name=f"op{i}")
                                   for i in range(npair)]
                            for gi, g in enumerate(gs):
                                for i in range(npair):
                                    nc.tensor.matmul(
                                        pss[i][:],
                                        ygq(g, p0 + i),
                                        wp_sb[g][:],
                                        start=(gi == 0), stop=(gi == len(gs) - 1))
                            if first:
                                for i in range(npair):
                                    tt = tg0 + p0 + i
                                    pt_t = part_pool.tile([128, OC], F32,
                                                          tag=f"part{b}_{tt}",
                                                          name=f"part{b}_{tt}")
                                    nc.vector.tensor_add(pt_t[:], pss[i][:], bp_sb[:])
                                    partial[(b, tt)] = pt_t
                            else:
                                # pair of token tiles -> one [128, 2*OC] SBUF
                                # tile -> one merged output-store DMA
                                ot = ot_pool.tile([128, npair * OC], F32,
                                                  tag="ot", name="ot")
                                for i in range(npair):
                                    tt = tg0 + p0 + i
                                    nc.vector.tensor_add(
                                        ot[:, i * OC:(i + 1) * OC], pss[i][:],
                                        partial[(b, tt)][:])
                                r0 = b * T + (tg0 + p0) * 128
                                nc.sync.dma_start(
                                    out[r0:r0 + npair * 128, :].rearrange(
                                        "(blk p) t -> p blk t", p=128),
                                    ot[:].rearrange("p (blk t) -> p blk t",
                                                    t=OC))

                for b in range(B):
                    attention_block(0, b)
                for b in range(B):
                    proj_half(b, 0, first=True)
                for b in range(B):
                    attention_block(1, b)
                    proj_half(b, 1, first=False)
    nc.compile()
    return nc


def _prep_inputs(x, W_attn, b_attn, W_proj, b_proj, cos, sin, core, B, T):
    """Host-side shard prep for one core."""
    BT = B * T
    xT = np.ascontiguousarray(x.reshape(BT, C).T).astype(np.float16)

    cols = []
    bvals = []
    for part in range(3):  # q, k, v
        for j in range(HL):
            h = 2 * core + j
            sl = slice(part * C + h * HD, part * C + (h + 1) * HD)
            cols.append(W_attn[:, sl])
            bvals.append(b_attn[sl])
    wqkv = np.concatenate(cols, axis=1).astype(np.float16)
    bqkv = np.concatenate(bvals).astype(np.float32).reshape(-1, 1)

    # RoPE tables: ctil[p, t] = cos[t, p//2]; stil[2i] = -sin, stil[2i+1] = +sin
    cosr = np.repeat(cos.T, 2, axis=0)  # [128, T]
    sinr = np.repeat(sin.T, 2, axis=0)
    sgn = np.where((np.arange(128) % 2) == 0, -1.0, 1.0)[:, None]
    ctil = cosr.astype(np.float16)
    stil = (sinr * sgn).astype(np.float16)

    wp_c = W_proj[:, core * OC:(core + 1) * OC].astype(np.float16)
    bpb = np.tile(b_proj[core * OC:(core + 1) * OC].astype(np.float32), (128, 1))
    ii, jj = np.mgrid[0:128, 0:128]
    cmask = np.where(jj <= ii, 0.0, MASK_VAL).astype(np.float32)
    return {
        "xT": xT, "wqkv": wqkv, "bqkv": bqkv, "ctil": ctil, "stil": stil,
        "wp": wp_c, "bpb": bpb, "cmask": cmask,
    }


@functools.lru_cache(maxsize=2)
def _built(B, T):
    return build(B=B, T=T)


_warmed = set()


def kernel(x, W_attn, b_attn, W_proj, b_proj, cos, sin):
    x = np.asarray(x, dtype=np.float32)
    W_attn = np.asarray(W_attn, dtype=np.float32)
    b_attn = np.asarray(b_attn, dtype=np.float32)
    W_proj = np.asarray(W_proj, dtype=np.float32)
    b_proj = np.asarray(b_proj, dtype=np.float32)
    cos = np.asarray(cos, dtype=np.float32)
    sin = np.asarray(sin, dtype=np.float32)

    B, T, Cv = x.shape
    assert Cv == C
    nc = _built(B, T)
    in_maps = [_prep_inputs(x, W_attn, b_attn, W_proj, b_proj, cos, sin, c, B, T)
               for c in range(N_CORES)]
    if (B, T) not in _warmed:
        # The very first execution of a freshly loaded NEFF has been observed
        # to deliver stale/uninitialized collective buffers; run once and
        # discard, then run for real.
        run_bass_kernel_spmd(nc, in_maps, core_ids=list(range(N_CORES)))
        _warmed.add((B, T))
    res = run_bass_kernel_spmd(nc, in_maps, core_ids=list(range(N_CORES)))
    outs = [res.results[c]["out"] for c in range(N_CORES)]
    full = np.concatenate(outs, axis=1)  # [BT, C]
    return full.reshape(B, T, C).astype(np.float32)



# revision 11
# speedup vs baseline: 1.1431x; 1.1234x over previous
"""Causal self-attention (RoPE) Trainium2 kernel, 8-way tensor-parallel.

Sharding (Megatron-style, zero-cost input distribution since every core
receives the full inputs): core c owns global heads {2c, 2c+1}.

Per core:
  1. qkv^T = W_slice^T @ x^T   (fp16 matmuls, fp32 psum), single pass over x^T;
     per token-chunk epilogue: bias add, RoPE on q/k (pair-swap DMA + 3 DVE
     ops), v PE-transposed to [token, d] layout
  2. Per (head, batch): causal flash-style attention
       scores psum [q,k] -> +mask -> exp (ACT, fused row-sum accum) ->
       normalize by 1/l (DVE) -> PE-transpose P-hat -> PV matmuls -> y^T [d,q]
  3. AllGather of y^T slices across the 8 cores, one per (head, batch),
     so projection work can start while later attention blocks still run
  4. Output projection vs the core's 256-column slice of W_proj in two
     accumulation halves (even heads -> fp32 partial in SBUF as soon as the
     even AGs land, odd heads + partial + bias -> fp32 out [tokens, 256])

Host side shards weights, builds RoPE/mask tables, and concatenates the
8 column slices into the final [B, T, C] output.
"""

import functools
import numpy as np

import concourse.bass as bass
import concourse.mybir as mybir
import concourse.tile as tile
from concourse import bacc
from concourse.bass_utils import run_bass_kernel_spmd
from concourse.masks import make_identity
from concourse.tile import add_dep_helper

F32 = mybir.dt.float32
F16 = mybir.dt.float16

N_CORES = 8
C = 2048           # model dim
H = 16             # total heads
HD = 128           # head dim
HL = 2             # heads per core
OC = C // N_CORES  # output cols per core (256)
SCALE = 1.0 / float(np.sqrt(HD))
MASK_VAL = -900.0  # additive pre-scale mask; exp arg ~ -80 -> underflows to 0


def build(B=2, T=2048, collective=True, n_cores=N_CORES):
    """Build the SPMD Bass program (identical on every core)."""
    BT = B * T
    NSTR = 3 * HL                  # qkv strips of 128 cols
    NCT = C // 128                 # contraction tiles
    NTCH = BT // 512               # token chunks for qkv
    NQC = T // 512                 # q chunks per (b, h)
    NTT = T // 128                 # token tiles per batch

    nc = bacc.Bacc(None, target_bir_lowering=False)
    xT = nc.dram_tensor("xT", [C, BT], F16, kind="ExternalInput")
    wqkv = nc.dram_tensor("wqkv", [C, NSTR * 128], F16, kind="ExternalInput")
    bqkv = nc.dram_tensor("bqkv", [NSTR * 128, 1], F32, kind="ExternalInput")
    ctil = nc.dram_tensor("ctil", [128, T], F16, kind="ExternalInput")
    stil = nc.dram_tensor("stil", [128, T], F16, kind="ExternalInput")
    wp = nc.dram_tensor("wp", [C, OC], F16, kind="ExternalInput")
    bpb = nc.dram_tensor("bpb", [128, OC], F32, kind="ExternalInput")
    cmask = nc.dram_tensor("cmask", [128, 128], F32, kind="ExternalInput")
    onec = nc.dram_tensor("onec", [128, 1], F16, kind="ExternalInput")
    out = nc.dram_tensor("out", [BT, OC], F32, kind="ExternalOutput")

    with tile.TileContext(nc) as tc:
        with (
            tc.tile_pool(name="big", bufs=1) as big,
            tc.tile_pool(name="dram", bufs=1, space="DRAM") as dram,
        ):
            # ---- persistent SBUF tensors ----
            qr = big.tile([128, HL * BT], F16, tag="qr")
            kr = big.tile([128, HL * BT], F16, tag="kr")
            v_sb = big.tile([128, HL * BT], F16, tag="v_sb")
            ct_sb = big.tile([128, T], F16, tag="ct")
            st_sb = big.tile([128, T], F16, tag="st")
            ident = big.tile([128, 128], F16, tag="ident")
            cm_sb = big.tile([128, 128], F32, tag="cm")
            bq_sb = big.tile([128, NSTR], F32, tag="bq")
            bp_sb = big.tile([128, OC], F32, tag="bp")
            ones_sb = big.tile([128, 1], F16, tag="ones")

            # DRAM bounce buffers: one AllGather per (local head j, batch b)
            # every (head, batch) gather is split into two column halves so each
            # projection half starts while the second half is still on the wire
            split_ag = T >= 1024
            nh = 2 if split_ag else 1
            hw_ = T // nh
            agin = {}
            agout = {}
            for j in range(HL):
                for b in range(B):
                    for h in range(nh):
                        agin[(j, b, h)] = dram.tile([128, hw_], F16,
                                                    name=f"agin{j}_{b}_{h}")
                        agout[(j, b, h)] = dram.tile([n_cores * 128, hw_], F16,
                                                     name=f"agout{j}_{b}_{h}")

            # ================= Phase A: QKV + RoPE + v-transpose =================
            with (
                tc.tile_pool(name="wq", bufs=1) as wq_pool,
                tc.tile_pool(name="xt", bufs=2) as xt_pool,
                tc.tile_pool(name="rope", bufs=2) as rope_pool,
                tc.tile_pool(name="stage", bufs=2) as stage_pool,
                tc.tile_pool(name="qkv_ps", bufs=3, space="PSUM") as qkv_ps,
                tc.tile_pool(name="vt_ps", bufs=4, space="PSUM") as vt_ps,
            ):
                # Merged multi-tile DMAs: one SBUF tile holds all NCT 128-row
                # blocks side by side; a single DMA with a 3-d access pattern
                # fills it. Quarters of the weight tensor interleave with the
                # first-chunk x halves so the first matmuls start early.
                w_all = wq_pool.tile([128, NCT * NSTR * 128], F16, tag="wall",
                                     name="wall")

                def w_slice(ctn, s):
                    off = ctn * NSTR * 128 + s * 128
                    return w_all[:, off:off + 128]

                def load_xt_chunk(tch, ctn0, ctn1, xt_t=None):
                    if xt_t is None:
                        xt_t = xt_pool.tile([128, NCT * 512], F16, tag="xtch",
                                            name="xtch")
                    src = xT[ctn0 * 128:ctn1 * 128,
                             tch * 512:(tch + 1) * 512]
                    nc.sync.dma_start(
                        xt_t[:, ctn0 * 512:ctn1 * 512].rearrange(
                            "p (c t) -> p c t", t=512),
                        src.rearrange("(c p) t -> p c t", p=128))
                    return xt_t

                def load_w(c0, c1):
                    nc.sync.dma_start(
                        w_all[:, c0 * NSTR * 128:c1 * NSTR * 128]
                        .rearrange("p (c t) -> p c t", t=NSTR * 128),
                        wqkv[c0 * 128:c1 * 128, :]
                        .rearrange("(c p) t -> p c t", p=128))

                # first chunk: x half, w quarters interleaved, x half, w half
                xt_first = load_xt_chunk(0, 0, NCT // 2)
                load_w(0, 4)
                load_w(4, 8)
                load_xt_chunk(0, NCT // 2, NCT, xt_t=xt_first)
                load_w(8, NCT)
                nc.sync.dma_start(
                    bq_sb[:, 0:NSTR].rearrange("p (c t) -> p c t", t=1),
                    bqkv[:, :].rearrange("(c p) t -> p c t", p=128))
                make_identity(nc, ident[:])

                for tch in range(NTCH):
                    if tch == min(1, NTCH - 1):
                        # constants land after the first xT burst is in flight
                        nc.sync.dma_start(ct_sb[:], ctil[:, :])
                        nc.sync.dma_start(st_sb[:], stil[:, :])
                        nc.sync.dma_start(cm_sb[:], cmask[:, :])
                        nc.sync.dma_start(bp_sb[:], bpb[:, :])
                        nc.sync.dma_start(ones_sb[:], onec[:, :])
                    tw = (tch * 512) % T        # token offset within batch
                    tok = slice(tw, tw + 512)
                    if tch == 0:
                        xts = xt_first
                    else:
                        xts = load_xt_chunk(tch, 0, NCT)
                    # combined q/k staging tile so the RoPE pair-swap is two
                    # strided DMAs per chunk instead of two per strip
                    qkstage = stage_pool.tile([128, 4 * 512], F16, tag="qkst",
                                              name="qkst")
                    vstg = {}
                    for s in range(NSTR):
                        ps = qkv_ps.tile([128, 512], F32, name="qkvps")
                        for ctn in range(NCT):
                            nc.tensor.matmul(
                                ps[:], w_slice(ctn, s),
                                xts[:, ctn * 512:(ctn + 1) * 512],
                                start=(ctn == 0), stop=(ctn == NCT - 1))
                        if s < 4:
                            st_t = qkstage[:, s * 512:(s + 1) * 512]
                        else:
                            st_t = stage_pool.tile([128, 512], F16,
                                                   tag=f"vs{s - 4}",
                                                   name=f"vs{s - 4}")
                            vstg[s - 4] = st_t
                        nc.scalar.activation(
                            st_t[:], ps[:], mybir.ActivationFunctionType.Identity,
                            bias=bq_sb[:, s:s + 1], scale=1.0)
                    # pair-swap of the whole q/k staging block (2 strided DMAs)
                    sw = rope_pool.tile([128, 4 * 512], F16, tag="swqk",
                                        name="swqk")
                    nc.sync.dma_start(sw[0:127:2, :], qkstage[1:128:2, :])
                    nc.sync.dma_start(sw[1:128:2, :], qkstage[0:127:2, :])
                    for j in range(HL):
                        # RoPE on q and k for this (j, tch)
                        for part, dst in ((j, qr), (2 + j, kr)):
                            st_t = qkstage[:, part * 512:(part + 1) * 512]
                            sw_t = sw[:, part * 512:(part + 1) * 512]
                            dstsl = dst[:, j * BT + tch * 512: j * BT + (tch + 1) * 512]
                            tmp = rope_pool.tile([128, 512], F16, tag=f"rt{part}",
                                                 name=f"rt{part}")
                            nc.vector.tensor_mul(dstsl, st_t, ct_sb[:, tok])
                            nc.vector.tensor_mul(tmp[:], sw_t, st_sb[:, tok])
                            nc.vector.tensor_add(dstsl, dstsl, tmp[:])
                        # v transpose for this (j, tch)
                        for blk in range(4):
                            tt = tch * 4 + blk
                            ps = vt_ps.tile([128, 128], F16, tag="vtp", name="vtp")
                            nc.tensor.transpose(
                                ps[:], vstg[j][:, blk * 128:(blk + 1) * 128],
                                ident[:])
                            nc.scalar.copy(
                                v_sb[:, j * BT + tt * 128: j * BT + (tt + 1) * 128],
                                ps[:])

            # ============ Phase B: attention + AG + interleaved projection ============
            #
            # Attention is computed in the transposed (S^T) orientation:
            # scores tiles [k, q] come straight off the PE with k on
            # partitions, so exp (ACT) writes P^T directly into SBUF and the
            # PV matmuls stream it with no PE transposes or DVE copies. The
            # softmax denominator is recovered with an M=1 ones-matmul that
            # accumulates column sums in PSUM; y^T columns are scaled by 1/l
            # on the way out.
            with (
                tc.tile_pool(name="pt", bufs=2) as pt_pool,
                tc.tile_pool(name="stat", bufs=2) as stat_pool,
                tc.tile_pool(name="yts", bufs=2) as yts_pool,
                tc.tile_pool(name="wpp", bufs=1) as wp_pool,
                tc.tile_pool(name="ygs", bufs=2) as ygs_pool,
                tc.tile_pool(name="part", bufs=1) as part_pool,
                tc.tile_pool(name="ot", bufs=2) as ot_pool,
                tc.tile_pool(name="sc_ps", bufs=3, space="PSUM") as sc_ps,
                tc.tile_pool(name="y_ps", bufs=2, space="PSUM") as y_ps,
                tc.tile_pool(name="l_ps", bufs=1, space="PSUM") as l_ps,
                tc.tile_pool(name="o_ps", bufs=1, space="PSUM") as o_ps,
            ):
                # W_proj rows are host-reordered parity-major: even heads'
                # 8x128 rows first, then odd heads'. Two merged DMAs.
                wp_all = wp_pool.tile([128, H * OC], F16, tag="wpall",
                                      name="wpall")
                for half in range(2):
                    nc.sync.dma_start(
                        wp_all[:, half * 8 * OC:(half + 1) * 8 * OC]
                        .rearrange("p (c t) -> p c t", t=OC),
                        wp[half * 8 * 128:(half + 1) * 8 * 128, :]
                        .rearrange("(c p) t -> p c t", p=128))

                def wp_sb(g):
                    blk = 8 * (g % 2) + g // 2
                    return wp_all[:, blk * OC:(blk + 1) * OC]

                partial = {}
                cc_insts = {}
                agin_dmas = {}
                hqc = max(1, NQC // nh)

                def emit_ag(j, b, h):
                    if collective:
                        cc = nc.gpsimd.collective_compute(
                            "AllGather",
                            mybir.AluOpType.bypass,
                            replica_groups=[list(range(n_cores))],
                            ins=[agin[(j, b, h)].opt()],
                            outs=[agout[(j, b, h)].opt()],
                        )
                    else:
                        cc = nc.sync.dma_start(agout[(j, b, h)][0:128, :],
                                               agin[(j, b, h)][:, :])
                    cci = cc.ins if hasattr(cc, "ins") else cc
                    for d in agin_dmas[(j, b)][h * hqc:(h + 1) * hqc]:
                        di = d.ins if hasattr(d, "ins") else d
                        add_dep_helper(cci, di,
                                       reason="collective reads agin after y")
                    cc_insts.setdefault((j, b), {})[h] = cci

                def attn_qc(j, b, qc):
                    """One flash unit: S^T tiles -> exp -> {l,PV} matmuls,
                    2-deep software pipeline on the score psum."""
                    base = j * BT + b * T
                    K = 4 * qc + 4
                    ptall = pt_pool.tile([128, 8192], F16, tag="ptall",
                                         name="ptall")
                    psy = y_ps.tile([128, 512], F32, tag="psy", name="psy")
                    lps = l_ps.tile([1, 512], F32, tag="lps", name="lps")
                    q_ap = qr[:, base + qc * 512: base + (qc + 1) * 512]

                    def consume(kt):
                        qs = max(0, kt - 4 * qc) * 128
                        pt_ap = ptall[:, kt * 512 + qs:(kt + 1) * 512]
                        nc.tensor.matmul(
                            lps[:, qs:512], ones_sb[:, 0:1], pt_ap,
                            start=(kt == 0), stop=(kt == K - 1))
                        nc.tensor.matmul(
                            psy[:, qs:512],
                            v_sb[:, base + kt * 128: base + (kt + 1) * 128],
                            pt_ap, start=(kt == 0), stop=(kt == K - 1))

                    for kt in range(K):
                        sps = sc_ps.tile([128, 512], F32, tag="sps", name="sps")
                        nc.tensor.matmul(
                            sps[:], kr[:, base + kt * 128: base + (kt + 1) * 128],
                            q_ap, start=True, stop=True)
                        m = kt - 4 * qc
                        if m >= 0:
                            # diagonal-band tile: transposed triangular mask
                            nc.vector.tensor_add(
                                sps[:, m * 128:(m + 1) * 128],
                                sps[:, m * 128:(m + 1) * 128], cm_sb[:])
                        nc.scalar.activation(
                            ptall[:, kt * 512:(kt + 1) * 512], sps[:],
                            mybir.ActivationFunctionType.Exp, scale=SCALE)
                        if kt >= 2:
                            consume(kt - 2)
                    consume(K - 2)
                    consume(K - 1)

                    rec = stat_pool.tile([1, 512], F32, tag="rec", name="rec")
                    nc.vector.reciprocal(rec[:], lps[0:1, :])
                    recb = stat_pool.tile([128, 512], F32, tag="recb",
                                          name="recb")
                    nc.gpsimd.partition_broadcast(recb[:], rec[:])
                    yt = yts_pool.tile([128, 512], F16, tag="yt", name="yt")
                    nc.vector.tensor_mul(yt[:], psy[:], recb[:])
                    d = nc.sync.dma_start(
                        agin[(j, b, qc // hqc)][:, (qc % hqc) * 512:
                                                (qc % hqc + 1) * 512], yt[:])
                    agin_dmas.setdefault((j, b), []).append(d)
                    if (qc + 1) % hqc == 0:
                        emit_ag(j, b, qc // hqc)

                def proj_group(b, par, tg0, first):
                    """One 4-token-tile projection unit for heads of parity
                    `par` (all in one agout buffer): 1 merged yg DMA + 2 psum
                    pair-groups.

                    first=True: psum + bias -> fp16 partial tiles in SBUF.
                    first=False: psum + partial -> merged output-store DMA.
                    """
                    gs = list(range(par, H, 2))
                    ng = min(4, NTT - tg0)
                    h = tg0 * 128 // hw_
                    ygall = ygs_pool.tile([128, 8 * ng * 128], F16,
                                          tag="ygall", name="ygall")
                    src_ap = agout[(par, b, h)][
                        :, tg0 * 128 - h * hw_:(tg0 + ng) * 128 - h * hw_]
                    ccdep = cc_insts[(par, b)][h]
                    d = nc.sync.dma_start(
                        ygall[:].rearrange("p (blk t) -> p blk t",
                                           t=ng * 128),
                        src_ap.rearrange("(blk p) t -> p blk t", p=128))
                    di = d.ins if hasattr(d, "ins") else d
                    add_dep_helper(di, ccdep,
                                   reason="proj reads agout after collective")

                    def ygq(g, i):
                        off = (g // 2) * ng * 128 + i * 128
                        return ygall[:, off:off + 128]

                    for p0 in range(0, ng, 2):
                        npair = min(2, ng - p0)
                        pss = [o_ps.tile([128, OC], F32, tag=f"op{i}",
                                         name=f"op{i}")
                               for i in range(npair)]
                        for gi, g in enumerate(gs):
                            for i in range(npair):
                                nc.tensor.matmul(
                                    pss[i][:], ygq(g, p0 + i), wp_sb(g),
                                    start=(gi == 0), stop=(gi == len(gs) - 1))
                        if first:
                            for i in range(npair):
                                tt = tg0 + p0 + i
                                pt_t = part_pool.tile([128, OC], F16,
                                                      tag=f"part{b}_{tt}",
                                                      name=f"part{b}_{tt}")
                                nc.vector.tensor_add(pt_t[:], pss[i][:], bp_sb[:])
                                partial[(b, tt)] = pt_t
                        else:
                            # pair of token tiles -> one [128, 2*OC] SBUF
                            # tile -> one merged output-store DMA
                            ot = ot_pool.tile([128, npair * OC], F32,
                                              tag="ot", name="ot")
                            for i in range(npair):
                                tt = tg0 + p0 + i
                                nc.vector.tensor_add(
                                    ot[:, i * OC:(i + 1) * OC], pss[i][:],
                                    partial[(b, tt)][:])
                            r0 = b * T + (tg0 + p0) * 128
                            nc.sync.dma_start(
                                out[r0:r0 + npair * 128, :].rearrange(
                                    "(blk p) t -> p blk t", p=128),
                                ot[:].rearrange("p (blk t) -> p blk t",
                                                t=OC))

                # Emission order: attention blocks (0,0) (1,0) run clean;
                # projection groups weave into (0,1) and (1,1) as soon as
                # their AllGather half is in flight, leaving only the last
                # four parity-1 groups after the final AG half.
                for qc in range(NQC):
                    attn_qc(0, 0, qc)
                for qc in range(NQC):
                    attn_qc(1, 0, qc)
                for qc in range(NQC):
                    attn_qc(0, 1, qc)
                    proj_group(0, 0, 8 * (qc // 2) + 4 * (qc % 2), first=True)
                for qc in range(NQC):
                    attn_qc(1, 1, qc)
                    proj_group(1, 0, 8 * (qc // 2) + 4 * (qc % 2), first=True)
                for tg0 in range(0, NTT, 4):
                    proj_group(0, 1, tg0, first=False)
                for tg0 in range(0, NTT, 4):
                    proj_group(1, 1, tg0, first=False)
    nc.compile()
    return nc


def _prep_inputs(x, W_attn, b_attn, W_proj, b_proj, cos, sin, core, B, T):
    """Host-side shard prep for one core."""
    BT = B * T
    xT = np.ascontiguousarray(x.reshape(BT, C).T).astype(np.float16)

    cols = []
    bvals = []
    for part in range(3):  # q, k, v
        for j in range(HL):
            h = 2 * core + j
            sl = slice(part * C + h * HD, part * C + (h + 1) * HD)
            cols.append(W_attn[:, sl])
            bvals.append(b_attn[sl])
    wqkv = np.concatenate(cols, axis=1).astype(np.float16)
    bqkv = np.concatenate(bvals).astype(np.float32).reshape(-1, 1)

    # RoPE tables: ctil[p, t] = cos[t, p//2]; stil[2i] = -sin, stil[2i+1] = +sin
    cosr = np.repeat(cos.T, 2, axis=0)  # [128, T]
    sinr = np.repeat(sin.T, 2, axis=0)
    sgn = np.where((np.arange(128) % 2) == 0, -1.0, 1.0)[:, None]
    ctil = cosr.astype(np.float16)
    stil = (sinr * sgn).astype(np.float16)

    # W_proj rows parity-major: even global heads' 128-row blocks first,
    # then odd heads' (matches the kernel's per-parity agout layout)
    worder = [g for g in range(H) if g % 2 == 0] + \
             [g for g in range(H) if g % 2 == 1]
    wp_c = np.concatenate(
        [W_proj[g * HD:(g + 1) * HD, core * OC:(core + 1) * OC]
         for g in worder], axis=0).astype(np.float16)
    bpb = np.tile(b_proj[core * OC:(core + 1) * OC].astype(np.float32), (128, 1))
    ii, jj = np.mgrid[0:128, 0:128]
    # transposed causal triangle for the S^T orientation: row=k, col=q,
    # masked (additive -900 pre-scale) where k > q
    cmask = np.where(ii <= jj, 0.0, MASK_VAL).astype(np.float32)
    onec = np.ones((128, 1), dtype=np.float16)
    return {
        "xT": xT, "wqkv": wqkv, "bqkv": bqkv, "ctil": ctil, "stil": stil,
        "wp": wp_c, "bpb": bpb, "cmask": cmask, "onec": onec,
    }


@functools.lru_cache(maxsize=2)
def _built(B, T):
    return build(B=B, T=T)


_warmed = set()


def kernel(x, W_attn, b_attn, W_proj, b_proj, cos, sin):
    x = np.asarray(x, dtype=np.float32)
    W_attn = np.asarray(W_attn, dtype=np.float32)
    b_attn = np.asarray(b_attn, dtype=np.float32)
    W_proj = np.asarray(W_proj, dtype=np.float32)
    b_proj = np.asarray(b_proj, dtype=np.float32)
    cos = np.asarray(cos, dtype=np.float32)
    sin = np.asarray(sin, dtype=np.float32)

    B, T, Cv = x.shape
    assert Cv == C
    nc = _built(B, T)
    in_maps = [_prep_inputs(x, W_attn, b_attn, W_proj, b_proj, cos, sin, c, B, T)
               for c in range(N_CORES)]
    if (B, T) not in _warmed:
        # The very first execution of a freshly loaded NEFF has been observed
        # to deliver stale/uninitialized collective buffers; run once and
        # discard, then run for real.
        run_bass_kernel_spmd(nc, in_maps, core_ids=list(range(N_CORES)))
        _warmed.add((B, T))
    res = run_bass_kernel_spmd(nc, in_maps, core_ids=list(range(N_CORES)))
    outs = [res.results[c]["out"] for c in range(N_CORES)]
    full = np.concatenate(outs, axis=1)  # [BT, C]
    return full.reshape(B, T, C).astype(np.float32)



# revision 13
# speedup vs baseline: 1.1693x; 1.0229x over previous
"""Causal self-attention (RoPE) Trainium2 kernel, 8-way tensor-parallel.

Sharding (Megatron-style, zero-cost input distribution since every core
receives the full inputs): core c owns global heads {2c, 2c+1}.

Per core:
  1. qkv^T = W_slice^T @ x^T   (fp16 matmuls, fp32 psum), one merged
     multi-tile DMA per 512-token chunk; per-chunk epilogue: bias add,
     RoPE on q/k (one pair-swap DMA pair per chunk + 3 DVE ops per
     (head, q/k)), v PE-transposed to [token, d] layout
  2. Per (head, batch): causal flash attention in the TRANSPOSED (S^T)
     orientation: scores tiles [k, q] come straight off the PE with k on
     partitions, exp (ACT) writes P^T directly into SBUF, and the PV
     matmuls stream it with no PE transposes. The softmax denominator is
     an M=1 ones-matmul accumulating column sums in PSUM; y^T columns are
     scaled by 1/l (reciprocal -> gpsimd partition-broadcast -> DVE mul).
  3. AllGather of y^T slices (one per head/batch/column-half) so
     projection starts while later attention blocks still run
  4. Output projection vs the core's 256-column slice of W_proj in two
     parity halves (even heads -> fp16 partials in SBUF, odd heads +
     partial + bias -> fp32 out), interleaved into the attention stream
     as PE filler work

Attention for batch 0 is interleaved into QKV chunks 4-7 (its qkv data
is complete after chunk 3), sharing one rotating PSUM pool between QKV
strip accumulators and score tiles to stay within 8 banks.

Host side shards weights, builds RoPE/mask tables, and concatenates the
8 column slices into the final [B, T, C] output.
"""

import functools
import numpy as np

import concourse.bass as bass
import concourse.mybir as mybir
import concourse.tile as tile
from concourse import bacc
from concourse.bass_utils import run_bass_kernel_spmd
from concourse.masks import make_identity
from concourse.tile import add_dep_helper

F32 = mybir.dt.float32
F16 = mybir.dt.float16

N_CORES = 8
C = 2048           # model dim
H = 16             # total heads
HD = 128           # head dim
HL = 2             # heads per core
OC = C // N_CORES  # output cols per core (256)
SCALE = 1.0 / float(np.sqrt(HD))
MASK_VAL = -900.0  # additive pre-scale mask; exp arg ~ -80 -> underflows to 0


def build(B=2, T=2048, collective=True, n_cores=N_CORES):
    """Build the SPMD Bass program (identical on every core)."""
    BT = B * T
    NSTR = 3 * HL                  # qkv strips of 128 cols
    NCT = C // 128                 # contraction tiles
    NTCH = BT // 512               # token chunks for qkv
    NQC = T // 512                 # q chunks per (b, h)
    NTT = T // 128                 # token tiles per batch

    nc = bacc.Bacc(None, target_bir_lowering=False)
    xT = nc.dram_tensor("xT", [C, BT], F16, kind="ExternalInput")
    wqkv = nc.dram_tensor("wqkv", [C, NSTR * 128], F16, kind="ExternalInput")
    bqkv = nc.dram_tensor("bqkv", [NSTR * 128, 1], F32, kind="ExternalInput")
    ctil = nc.dram_tensor("ctil", [128, T], F16, kind="ExternalInput")
    stil = nc.dram_tensor("stil", [128, T], F16, kind="ExternalInput")
    wp = nc.dram_tensor("wp", [C, OC], F16, kind="ExternalInput")
    bpb = nc.dram_tensor("bpb", [128, OC], F32, kind="ExternalInput")
    cmask = nc.dram_tensor("cmask", [128, 128], F32, kind="ExternalInput")
    onec = nc.dram_tensor("onec", [128, 1], F16, kind="ExternalInput")
    out = nc.dram_tensor("out", [BT, OC], F32, kind="ExternalOutput")

    with tile.TileContext(nc) as tc:
        with (
            tc.tile_pool(name="big", bufs=1) as big,
            tc.tile_pool(name="dram", bufs=1, space="DRAM") as dram,
            tc.tile_pool(name="pt", bufs=2) as pt_pool,
            tc.tile_pool(name="stat", bufs=2) as stat_pool,
            tc.tile_pool(name="yts", bufs=2) as yts_pool,
            # one rotating psum pool shared by QKV strip accumulators and
            # attention score tiles (same [128, 512] f32 shape/tag)
            tc.tile_pool(name="mm_ps", bufs=3, space="PSUM") as mm_ps,
            tc.tile_pool(name="y_ps", bufs=2, space="PSUM") as y_ps,
            tc.tile_pool(name="l_ps", bufs=1, space="PSUM") as l_ps,
        ):
            # ---- persistent SBUF tensors ----
            qr = big.tile([128, HL * BT], F16, tag="qr")
            kr = big.tile([128, HL * BT], F16, tag="kr")
            v_sb = big.tile([128, HL * BT], F16, tag="v_sb")
            ct_sb = big.tile([128, T], F16, tag="ct")
            st_sb = big.tile([128, T], F16, tag="st")
            ident = big.tile([128, 128], F16, tag="ident")
            cm_sb = big.tile([128, 128], F32, tag="cm")
            bq_sb = big.tile([128, NSTR], F32, tag="bq")
            bp_sb = big.tile([128, OC], F32, tag="bp")
            ones_sb = big.tile([128, 1], F16, tag="ones")

            # DRAM bounce buffers: one AllGather per (local head, batch,
            # column half) so projection starts on the first half while the
            # second is still on the wire
            split_ag = T >= 1024
            nh = 2 if split_ag else 1
            hw_ = T // nh
            hqc = max(1, NQC // nh)
            agin = {}
            agout = {}
            for j in range(HL):
                for b in range(B):
                    for h in range(nh):
                        agin[(j, b, h)] = dram.tile([128, hw_], F16,
                                                    name=f"agin{j}_{b}_{h}")
                        agout[(j, b, h)] = dram.tile([n_cores * 128, hw_], F16,
                                                     name=f"agout{j}_{b}_{h}")

            partial = {}
            cc_insts = {}
            agin_dmas = {}

            def emit_ag(j, b, h):
                if collective:
                    cc = nc.gpsimd.collective_compute(
                        "AllGather",
                        mybir.AluOpType.bypass,
                        replica_groups=[list(range(n_cores))],
                        ins=[agin[(j, b, h)].opt()],
                        outs=[agout[(j, b, h)].opt()],
                    )
                else:
                    cc = nc.sync.dma_start(agout[(j, b, h)][0:128, :],
                                           agin[(j, b, h)][:, :])
                cci = cc.ins if hasattr(cc, "ins") else cc
                for d in agin_dmas[(j, b)][h * hqc:(h + 1) * hqc]:
                    di = d.ins if hasattr(d, "ins") else d
                    add_dep_helper(cci, di,
                                   reason="collective reads agin after y")
                cc_insts.setdefault((j, b), {})[h] = cci

            def attn_qc(j, b, qc, filler=None):
                """One flash unit in S^T orientation: score tiles [k, q] ->
                exp -> {l ones-matmul, PV matmul}, 2-deep software pipeline
                on the score psum. `filler` emits independent PE work after
                the first two score matmuls to cover the exp latency."""
                base = j * BT + b * T
                K = 4 * qc + 4
                ptall = pt_pool.tile([128, 8192], F16, tag="ptall",
                                     name="ptall")
                psy = y_ps.tile([128, 512], F32, tag="psy", name="psy")
                lps = l_ps.tile([1, 512], F32, tag="lps", name="lps")
                q_ap = qr[:, base + qc * 512: base + (qc + 1) * 512]

                def consume(kt):
                    qs = max(0, kt - 4 * qc) * 128
                    pt_ap = ptall[:, kt * 512 + qs:(kt + 1) * 512]
                    nc.tensor.matmul(
                        lps[:, qs:512], ones_sb[:, 0:1], pt_ap,
                        start=(kt == 0), stop=(kt == K - 1))
                    nc.tensor.matmul(
                        psy[:, qs:512],
                        v_sb[:, base + kt * 128: base + (kt + 1) * 128],
                        pt_ap, start=(kt == 0), stop=(kt == K - 1))

                for kt in range(K):
                    sps = mm_ps.tile([128, 512], F32, tag="ps512", name="sps")
                    nc.tensor.matmul(
                        sps[:], kr[:, base + kt * 128: base + (kt + 1) * 128],
                        q_ap, start=True, stop=True)
                    m = kt - 4 * qc
                    if m >= 0:
                        # diagonal-band tile: transposed triangular mask
                        nc.vector.tensor_add(
                            sps[:, m * 128:(m + 1) * 128],
                            sps[:, m * 128:(m + 1) * 128], cm_sb[:])
                    nc.scalar.activation(
                        ptall[:, kt * 512:(kt + 1) * 512], sps[:],
                        mybir.ActivationFunctionType.Exp, scale=SCALE)
                    if kt == 1 and filler is not None:
                        filler()
                    if kt >= 2:
                        consume(kt - 2)
                consume(K - 2)
                consume(K - 1)

                rec = stat_pool.tile([1, 512], F32, tag="rec", name="rec")
                nc.vector.reciprocal(rec[:], lps[0:1, :])
                recb = stat_pool.tile([128, 512], F32, tag="recb",
                                      name="recb")
                nc.gpsimd.partition_broadcast(recb[:], rec[:])
                yt = yts_pool.tile([128, 512], F16, tag="yt", name="yt")
                nc.vector.tensor_mul(yt[:], psy[:], recb[:])
                d = nc.gpsimd.dma_start(
                    agin[(j, b, qc // hqc)][:, (qc % hqc) * 512:
                                            (qc % hqc + 1) * 512], yt[:])
                agin_dmas.setdefault((j, b), []).append(d)
                if (qc + 1) % hqc == 0:
                    emit_ag(j, b, qc // hqc)

            # ================= Phase A: QKV + RoPE + v-transpose ============
            with (
                tc.tile_pool(name="wq", bufs=1) as wq_pool,
                tc.tile_pool(name="xt", bufs=2) as xt_pool,
                tc.tile_pool(name="rope", bufs=2) as rope_pool,
                tc.tile_pool(name="stage", bufs=2) as stage_pool,
                tc.tile_pool(name="vt_ps", bufs=2, space="PSUM") as vt_ps,
            ):
                # Merged multi-tile DMAs: one SBUF tile holds all NCT 128-row
                # blocks side by side; a single DMA with a 3-d access pattern
                # fills it. Quarters of the weight tensor interleave with the
                # first-chunk x halves so the first matmuls start early.
                w_all = wq_pool.tile([128, NCT * NSTR * 128], F16, tag="wall",
                                     name="wall")

                def w_slice(ctn, s):
                    off = ctn * NSTR * 128 + s * 128
                    return w_all[:, off:off + 128]

                def load_xt_chunk(tch, ctn0, ctn1, xt_t=None):
                    if xt_t is None:
                        xt_t = xt_pool.tile([128, NCT * 512], F16, tag="xtch",
                                            name="xtch")
                    src = xT[ctn0 * 128:ctn1 * 128,
                             tch * 512:(tch + 1) * 512]
                    nc.sync.dma_start(
                        xt_t[:, ctn0 * 512:ctn1 * 512].rearrange(
                            "p (c t) -> p c t", t=512),
                        src.rearrange("(c p) t -> p c t", p=128))
                    return xt_t

                def load_w(c0, c1):
                    nc.sync.dma_start(
                        w_all[:, c0 * NSTR * 128:c1 * NSTR * 128]
                        .rearrange("p (c t) -> p c t", t=NSTR * 128),
                        wqkv[c0 * 128:c1 * 128, :]
                        .rearrange("(c p) t -> p c t", p=128))

                # first chunk: x half, w quarters interleaved, x half, w half
                xt_first = load_xt_chunk(0, 0, NCT // 2)
                load_w(0, 4)
                load_w(4, 8)
                load_xt_chunk(0, NCT // 2, NCT, xt_t=xt_first)
                load_w(8, NCT)
                nc.sync.dma_start(
                    bq_sb[:, 0:NSTR].rearrange("p (c t) -> p c t", t=1),
                    bqkv[:, :].rearrange("(c p) t -> p c t", p=128))
                make_identity(nc, ident[:])

                # interleave schedule: after qkv for batch 0 lands (chunk 3),
                # its attention blocks weave into chunks 4..7 using the last
                # two qkv strips of each chunk as pipeline filler
                inter = {}
                if NTCH == 8 and NQC == 4:
                    inter = {4: [(0, 0, 0), (0, 0, 1)],
                             5: [(0, 0, 2), (0, 0, 3)],
                             6: [(1, 0, 0), (1, 0, 1)],
                             7: [(1, 0, 2), (1, 0, 3)]}

                for tch in range(NTCH):
                    if tch == min(1, NTCH - 1):
                        # constants land after the first xT burst is in flight
                        nc.sync.dma_start(ct_sb[:], ctil[:, :])
                        nc.sync.dma_start(st_sb[:], stil[:, :])
                        nc.sync.dma_start(cm_sb[:], cmask[:, :])
                        nc.sync.dma_start(bp_sb[:], bpb[:, :])
                        nc.sync.dma_start(ones_sb[:], onec[:, :])
                    tw = (tch * 512) % T        # token offset within batch
                    tok = slice(tw, tw + 512)
                    if tch == 0:
                        xts = xt_first
                    else:
                        xts = load_xt_chunk(tch, 0, NCT)
                    # combined q/k staging tile so the RoPE pair-swap is two
                    # strided DMAs per chunk instead of two per strip
                    qkstage = stage_pool.tile([128, 4 * 512], F16, tag="qkst",
                                              name="qkst")
                    vstg = {}

                    def emit_strip(s):
                        ps = mm_ps.tile([128, 512], F32, tag="ps512",
                                        name="qkvps")
                        for ctn in range(NCT):
                            nc.tensor.matmul(
                                ps[:], w_slice(ctn, s),
                                xts[:, ctn * 512:(ctn + 1) * 512],
                                start=(ctn == 0), stop=(ctn == NCT - 1))
                        if s < 4:
                            st_t = qkstage[:, s * 512:(s + 1) * 512]
                        else:
                            st_t = stage_pool.tile([128, 512], F16,
                                                   tag=f"vs{s - 4}",
                                                   name=f"vs{s - 4}")
                            vstg[s - 4] = st_t
                        nc.scalar.activation(
                            st_t[:], ps[:], mybir.ActivationFunctionType.Identity,
                            bias=bq_sb[:, s:s + 1], scale=1.0)

                    units = inter.get(tch, [])
                    for s in range(4):
                        emit_strip(s)
                    if units:
                        attn_qc(*units[0], filler=lambda: emit_strip(4))
                        attn_qc(*units[1], filler=lambda: emit_strip(5))
                    else:
                        emit_strip(4)
                        emit_strip(5)
                    # pair-swap of the whole q/k staging block (2 strided DMAs)
                    sw = rope_pool.tile([128, 4 * 512], F16, tag="swqk",
                                        name="swqk")
                    nc.sync.dma_start(sw[0:127:2, :], qkstage[1:128:2, :])
                    nc.sync.dma_start(sw[1:128:2, :], qkstage[0:127:2, :])
                    for j in range(HL):
                        # RoPE on q and k for this (j, tch)
                        for part, dst in ((j, qr), (2 + j, kr)):
                            st_t = qkstage[:, part * 512:(part + 1) * 512]
                            sw_t = sw[:, part * 512:(part + 1) * 512]
                            dstsl = dst[:, j * BT + tch * 512: j * BT + (tch + 1) * 512]
                            tmp = rope_pool.tile([128, 512], F16, tag=f"rt{j}",
                                                 name=f"rt{j}")
                            nc.vector.tensor_mul(dstsl, st_t, ct_sb[:, tok])
                            nc.vector.tensor_mul(tmp[:], sw_t, st_sb[:, tok])
                            nc.vector.tensor_add(dstsl, dstsl, tmp[:])
                        # v transpose for this (j, tch)
                        for blk in range(4):
                            tt = tch * 4 + blk
                            ps = vt_ps.tile([128, 128], F16, tag="vtp", name="vtp")
                            nc.tensor.transpose(
                                ps[:], vstg[j][:, blk * 128:(blk + 1) * 128],
                                ident[:])
                            nc.scalar.copy(
                                v_sb[:, j * BT + tt * 128: j * BT + (tt + 1) * 128],
                                ps[:])

            # ============ Phase B: remaining attention + projection =========
            with (
                tc.tile_pool(name="wpp", bufs=1) as wp_pool,
                tc.tile_pool(name="ygs", bufs=3) as ygs_pool,
                tc.tile_pool(name="part", bufs=1) as part_pool,
                tc.tile_pool(name="ot", bufs=2) as ot_pool,
                tc.tile_pool(name="o_ps", bufs=1, space="PSUM") as o_ps,
            ):
                # W_proj rows are host-reordered parity-major: even heads'
                # 8x128 rows first, then odd heads'. Two merged DMAs.
                wp_all = wp_pool.tile([128, H * OC], F16, tag="wpall",
                                      name="wpall")
                for half in range(2):
                    nc.sync.dma_start(
                        wp_all[:, half * 8 * OC:(half + 1) * 8 * OC]
                        .rearrange("p (c t) -> p c t", t=OC),
                        wp[half * 8 * 128:(half + 1) * 8 * 128, :]
                        .rearrange("(c p) t -> p c t", p=128))

                def wp_sb(g):
                    blk = 8 * (g % 2) + g // 2
                    return wp_all[:, blk * OC:(blk + 1) * OC]

                def proj_group(b, par, tg0, first):
                    """One 4-token-tile projection unit for heads of parity
                    `par` (all in one agout buffer): 1 merged yg DMA + 2 psum
                    pair-groups.

                    first=True: psum + bias -> fp16 partial tiles in SBUF.
                    first=False: psum + partial -> merged output-store DMA.
                    """
                    gs = list(range(par, H, 2))
                    ng = min(4, NTT - tg0)
                    h = tg0 * 128 // hw_
                    ygall = ygs_pool.tile([128, 8 * ng * 128], F16,
                                          tag="ygall", name="ygall")
                    src_ap = agout[(par, b, h)][
                        :, tg0 * 128 - h * hw_:(tg0 + ng) * 128 - h * hw_]
                    ccdep = cc_insts[(par, b)][h]
                    d = nc.sync.dma_start(
                        ygall[:].rearrange("p (blk t) -> p blk t",
                                           t=ng * 128),
                        src_ap.rearrange("(blk p) t -> p blk t", p=128))
                    di = d.ins if hasattr(d, "ins") else d
                    add_dep_helper(di, ccdep,
                                   reason="proj reads agout after collective")

                    def ygq(g, i):
                        off = (g // 2) * ng * 128 + i * 128
                        return ygall[:, off:off + 128]

                    for p0 in range(0, ng, 2):
                        npair = min(2, ng - p0)
                        pss = [o_ps.tile([128, OC], F32, tag=f"op{i}",
                                         name=f"op{i}")
                               for i in range(npair)]
                        for gi, g in enumerate(gs):
                            for i in range(npair):
                                nc.tensor.matmul(
                                    pss[i][:], ygq(g, p0 + i), wp_sb(g),
                                    start=(gi == 0), stop=(gi == len(gs) - 1))
                        if first:
                            for i in range(npair):
                                tt = tg0 + p0 + i
                                pt_t = part_pool.tile([128, OC], F16,
                                                      tag=f"part{b}_{tt}",
                                                      name=f"part{b}_{tt}")
                                nc.vector.tensor_add(pt_t[:], pss[i][:], bp_sb[:])
                                partial[(b, tt)] = pt_t
                        else:
                            # pair of token tiles -> one [128, 2*OC] SBUF
                            # tile -> one merged output-store DMA
                            ot = ot_pool.tile([128, npair * OC], F32,
                                              tag="ot", name="ot")
                            for i in range(npair):
                                tt = tg0 + p0 + i
                                nc.vector.tensor_add(
                                    ot[:, i * OC:(i + 1) * OC], pss[i][:],
                                    partial[(b, tt)][:])
                            r0 = b * T + (tg0 + p0) * 128
                            nc.sync.dma_start(
                                out[r0:r0 + npair * 128, :].rearrange(
                                    "(blk p) t -> p blk t", p=128),
                                ot[:].rearrange("p (blk t) -> p blk t",
                                                t=OC))

                # Emission order: batch-1 attention blocks carry projection
                # groups as filler (even-parity partials during (0,1), odd
                # batch-0 finals + even batch-1 partials during (1,1)),
                # leaving only the last parity-1 groups after the final AG.
                def tg_of(qc):
                    return 8 * (qc // 2) + 4 * (qc % 2)

                # fallback: any batch-0 units not interleaved into the chunks
                done_b0 = {u for us in inter.values() for u in us}
                for j in range(HL):
                    for qc in range(NQC):
                        if (j, 0, qc) not in done_b0:
                            attn_qc(j, 0, qc)

                for qc in range(NQC):
                    attn_qc(0, 1, qc, filler=functools.partial(
                        proj_group, 0, 0, tg_of(qc), True))
                for qc in range(NQC):
                    attn_qc(1, 1, qc, filler=functools.partial(
                        proj_group, 1, 0, tg_of(qc), True))
                    proj_group(0, 1, tg_of(qc), first=False)
                for tg0 in range(0, NTT, 4):
                    proj_group(1, 1, tg0, first=False)
    nc.compile()
    return nc


def _prep_inputs(x, W_attn, b_attn, W_proj, b_proj, cos, sin, core, B, T):
    """Host-side shard prep for one core."""
    BT = B * T
    xT = np.ascontiguousarray(x.reshape(BT, C).T).astype(np.float16)

    cols = []
    bvals = []
    for part in range(3):  # q, k, v
        for j in range(HL):
            h = 2 * core + j
            sl = slice(part * C + h * HD, part * C + (h + 1) * HD)
            cols.append(W_attn[:, sl])
            bvals.append(b_attn[sl])
    wqkv = np.concatenate(cols, axis=1).astype(np.float16)
    bqkv = np.concatenate(bvals).astype(np.float32).reshape(-1, 1)

    # RoPE tables: ctil[p, t] = cos[t, p//2]; stil[2i] = -sin, stil[2i+1] = +sin
    cosr = np.repeat(cos.T, 2, axis=0)  # [128, T]
    sinr = np.repeat(sin.T, 2, axis=0)
    sgn = np.where((np.arange(128) % 2) == 0, -1.0, 1.0)[:, None]
    ctil = cosr.astype(np.float16)
    stil = (sinr * sgn).astype(np.float16)

    # W_proj rows parity-major: even global heads' 128-row blocks first,
    # then odd heads' (matches the kernel's per-parity agout layout)
    worder = [g for g in range(H) if g % 2 == 0] + \
             [g for g in range(H) if g % 2 == 1]
    wp_c = np.concatenate(
        [W_proj[g * HD:(g + 1) * HD, core * OC:(core + 1) * OC]
         for g in worder], axis=0).astype(np.float16)
    bpb = np.tile(b_proj[core * OC:(core + 1) * OC].astype(np.float32), (128, 1))
    ii, jj = np.mgrid[0:128, 0:128]
    # transposed causal triangle for the S^T orientation: row=k, col=q,
    # masked (additive -900 pre-scale) where k > q
    cmask = np.where(ii <= jj, 0.0, MASK_VAL).astype(np.float32)
    onec = np.ones((128, 1), dtype=np.float16)
    return {
        "xT": xT, "wqkv": wqkv, "bqkv": bqkv, "ctil": ctil, "stil": stil,
        "wp": wp_c, "bpb": bpb, "cmask": cmask, "onec": onec,
    }


@functools.lru_cache(maxsize=2)
def _built(B, T):
    return build(B=B, T=T)


_warmed = set()


def kernel(x, W_attn, b_attn, W_proj, b_proj, cos, sin):
    x = np.asarray(x, dtype=np.float32)
    W_attn = np.asarray(W_attn, dtype=np.float32)
    b_attn = np.asarray(b_attn, dtype=np.float32)
    W_proj = np.asarray(W_proj, dtype=np.float32)
    b_proj = np.asarray(b_proj, dtype=np.float32)
    cos = np.asarray(cos, dtype=np.float32)
    sin = np.asarray(sin, dtype=np.float32)

    B, T, Cv = x.shape
    assert Cv == C
    nc = _built(B, T)
    in_maps = [_prep_inputs(x, W_attn, b_attn, W_proj, b_proj, cos, sin, c, B, T)
               for c in range(N_CORES)]
    if (B, T) not in _warmed:
        # The very first execution of a freshly loaded NEFF has been observed
        # to deliver stale/uninitialized collective buffers; run once and
        # discard, then run for real.
        run_bass_kernel_spmd(nc, in_maps, core_ids=list(range(N_CORES)))
        _warmed.add((B, T))
    res = run_bass_kernel_spmd(nc, in_maps, core_ids=list(range(N_CORES)))
    outs = [res.results[c]["out"] for c in range(N_CORES)]
    full = np.concatenate(outs, axis=1)  # [BT, C]
    return full.reshape(B, T, C).astype(np.float32)
